# revision 1
# baseline (speedup 1.0000x reference)
"""Trainium2 Bass kernel for nn_Antecedents: fuzzy-rule antecedent activations.

Computes out[n, r] = prod_v memberships[v, n, set_v(r)] over the full
Cartesian product of fuzzy sets (R = 4**6 = 4096 rules), for N = 16384
samples, data-parallel over 8 NeuronCores (2048 samples per core).

Per-core layout: sample n = p*16 + j (p = SBUF partition 0..127,
j = 0..15).  The chained outer product is built per j-group from the
LAST variable backwards so every expansion step is a contiguous
tensor_scalar multiply with a per-partition scalar:

    acc_{k+1}[:, s*L:(s+1)*L] = acc_k[:, 0:L] * X_v[:, j*4+s]

which on fp32/SBUF runs in the DVE 2x perf mode.  The final 4x1024
expansion is split across VectorE / ScalarE / GpSimd so it hides under
the output-write DMA (32 MB/core, the memory-bound roofline).
"""

import numpy as np
from contextlib import ExitStack

import concourse.bass as bass
import concourse.tile as tile
from concourse import bacc, mybir
from concourse.bass_utils import run_bass_kernel_spmd

N_VARS = 6
N_FULL = 16384
N_SETS = 4
N_CORES = 8
N_SHARD = N_FULL // N_CORES  # 2048
P = 128
J = N_SHARD // P             # 16 samples per partition
R = N_SETS ** N_VARS         # 4096
JPAIR = 2                    # j-groups per output tile / output DMA
F32 = mybir.dt.float32

LAST_RESULTS = None
_CACHE = {}


def build_nc():
    nc = bacc.Bacc(
        "TRN2", target_bir_lowering=False, debug=False, num_devices=N_CORES
    )
    m = nc.dram_tensor(
        "memberships", [N_VARS, N_SHARD, N_SETS], F32, kind="ExternalInput"
    ).ap()
    out = nc.dram_tensor("out", [N_SHARD, R], F32, kind="ExternalOutput").ap()
    out_v = out.rearrange("(p f) r -> p (f r)", p=P)  # [128, J*R]

    with tile.TileContext(nc) as tc, ExitStack() as ctx:
        xpool = ctx.enter_context(tc.tile_pool(name="x", bufs=1))
        spool = ctx.enter_context(tc.tile_pool(name="scratch", bufs=2))
        opool = ctx.enter_context(tc.tile_pool(name="out", bufs=2))

        # X[v]: [128, 64] f32, column j*4 + s  <-  memberships[v, p*16+j, s]
        # (256 B contiguous per partition in DRAM -> one clean DMA per var)
        X = []
        for v in range(N_VARS):
            xv = xpool.tile([P, J * N_SETS], F32, tag=f"x{v}")
            nc.sync.dma_start(
                out=xv[:], in_=m[v].rearrange("(p f) s -> p (f s)", p=P)
            )
            X.append(xv)

        def sc(v, j, s):
            c = j * N_SETS + s
            return X[v][:, c : c + 1]

        for t in range(J // JPAIR):
            ot = opool.tile([P, JPAIR * R], F32, tag="ot")
            for jj in range(JPAIR):
                j = t * JPAIR + jj
                a16 = spool.tile([P, 16], F32, tag="a16")
                for s in range(N_SETS):
                    nc.vector.tensor_scalar_mul(
                        a16[:, 4 * s : 4 * (s + 1)],
                        X[5][:, j * 4 : (j + 1) * 4],
                        sc(4, j, s),
                    )
                a64 = spool.tile([P, 64], F32, tag="a64")
                for s in range(N_SETS):
                    nc.vector.tensor_scalar_mul(
                        a64[:, 16 * s : 16 * (s + 1)], a16[:], sc(3, j, s)
                    )
                a256 = spool.tile([P, 256], F32, tag="a256")
                for s in range(N_SETS):
                    nc.vector.tensor_scalar_mul(
                        a256[:, 64 * s : 64 * (s + 1)], a64[:], sc(2, j, s)
                    )
                a1024 = spool.tile([P, 1024], F32, tag="a1024")
                for s in range(N_SETS):
                    nc.vector.tensor_scalar_mul(
                        a1024[:, 256 * s : 256 * (s + 1)], a256[:], sc(1, j, s)
                    )
                b = jj * R
                # Final expansion: 4 x [128, 1024] split across engines.
                nc.vector.tensor_scalar_mul(
                    ot[:, b : b + 1024], a1024[:], sc(0, j, 0)
                )
                nc.scalar.activation(
                    ot[:, b + 1024 : b + 2048],
                    a1024[:],
                    mybir.ActivationFunctionType.Copy,
                    scale=sc(0, j, 1),
                )
                nc.scalar.activation(
                    ot[:, b + 2048 : b + 3072],
                    a1024[:],
                    mybir.ActivationFunctionType.Copy,
                    scale=sc(0, j, 2),
                )
                nc.gpsimd.tensor_scalar_mul(
                    ot[:, b + 3072 : b + 4096], a1024[:], sc(0, j, 3)
                )
            cols = t * JPAIR * R
            nc.sync.dma_start(out=out_v[:, cols : cols + JPAIR * R], in_=ot[:])

    nc.compile()
    return nc


def _get_nc():
    if "nc" not in _CACHE:
        _CACHE["nc"] = build_nc()
    return _CACHE["nc"]


def kernel(memberships):
    global LAST_RESULTS
    m = np.ascontiguousarray(np.asarray(memberships, dtype=np.float32))
    assert m.shape == (N_VARS, N_FULL, N_SETS), m.shape
    nc = _get_nc()
    shards = np.split(m, N_CORES, axis=1)
    in_maps = [{"memberships": np.ascontiguousarray(s)} for s in shards]
    res = run_bass_kernel_spmd(nc, in_maps, core_ids=list(range(N_CORES)))
    LAST_RESULTS = res
    return np.concatenate(
        [res.results[i]["out"] for i in range(N_CORES)], axis=0
    )


# revision 3
# speedup vs baseline: 2.9400x; 2.9400x over previous
"""Trainium2 Bass kernel for nn_Antecedents: fuzzy-rule antecedent activations.

Computes out[n, r] = prod_v memberships[v, n, set_v(r)] over the full
Cartesian product of fuzzy sets (R = 4**6 = 4096 rules), for N = 16384
samples, data-parallel over 8 NeuronCores (2048 samples per core).

Per-core layout: sample n = p*16 + j (p = SBUF partition 0..127,
j = 0..15).  The chained outer product is built per j-group from the
LAST variable backwards so every expansion step is a contiguous
tensor_scalar multiply with a per-partition scalar:

    acc_{k+1}[:, s*L:(s+1)*L] = acc_k[:, 0:L] * X_v[:, j*4+s]

which on fp32/SBUF runs in the DVE 2x perf mode.  The final 4x1024
expansion is split across VectorE / ScalarE / GpSimd so it hides under
the output-write DMA (32 MB/core, the memory-bound roofline).
"""

import numpy as np
from contextlib import ExitStack

import concourse.bass as bass
import concourse.tile as tile
from concourse import bacc, mybir
from concourse.bass_utils import run_bass_kernel_spmd

N_VARS = 6
N_FULL = 16384
N_SETS = 4
N_CORES = 8
N_SHARD = N_FULL // N_CORES  # 2048
P = 128
J = N_SHARD // P             # 16 samples per partition
R = N_SETS ** N_VARS         # 4096
JPAIR = 2                    # j-groups per output tile / output DMA
F32 = mybir.dt.float32

LAST_RESULTS = None
_CACHE = {}


def build_nc():
    nc = bacc.Bacc(
        "TRN2", target_bir_lowering=False, debug=False, num_devices=N_CORES
    )
    m = nc.dram_tensor(
        "memberships", [N_VARS, N_SHARD, N_SETS], F32, kind="ExternalInput"
    ).ap()
    out = nc.dram_tensor("out", [N_SHARD, R], F32, kind="ExternalOutput").ap()
    out_v = out.rearrange("(p f) r -> p (f r)", p=P)  # [128, J*R]

    with tile.TileContext(nc) as tc, ExitStack() as ctx:
        xpool = ctx.enter_context(tc.tile_pool(name="x", bufs=1))
        spool = ctx.enter_context(tc.tile_pool(name="scratch", bufs=2))
        opool = ctx.enter_context(tc.tile_pool(name="out", bufs=3))

        # X[v]: [128, 64] f32, column j*4 + s  <-  memberships[v, p*16+j, s]
        # (256 B contiguous per partition in DRAM -> one clean DMA per var)
        X = []
        for v in range(N_VARS):
            xv = xpool.tile([P, J * N_SETS], F32, tag=f"x{v}")
            nc.sync.dma_start(
                out=xv[:], in_=m[v].rearrange("(p f) s -> p (f s)", p=P)
            )
            X.append(xv)

        def sc(v, j, s):
            c = j * N_SETS + s
            return X[v][:, c : c + 1]

        for t in range(J // JPAIR):
            ot = opool.tile([P, JPAIR * R], F32, tag="ot")
            for jj in range(JPAIR):
                j = t * JPAIR + jj
                a16 = spool.tile([P, 16], F32, tag="a16")
                for s in range(N_SETS):
                    nc.vector.tensor_scalar_mul(
                        a16[:, 4 * s : 4 * (s + 1)],
                        X[5][:, j * 4 : (j + 1) * 4],
                        sc(4, j, s),
                    )
                a64 = spool.tile([P, 64], F32, tag="a64")
                for s in range(N_SETS):
                    nc.vector.tensor_scalar_mul(
                        a64[:, 16 * s : 16 * (s + 1)], a16[:], sc(3, j, s)
                    )
                a256 = spool.tile([P, 256], F32, tag="a256")
                for s in range(N_SETS):
                    nc.vector.tensor_scalar_mul(
                        a256[:, 64 * s : 64 * (s + 1)], a64[:], sc(2, j, s)
                    )
                a1024 = spool.tile([P, 1024], F32, tag="a1024")
                for s in range(N_SETS):
                    nc.vector.tensor_scalar_mul(
                        a1024[:, 256 * s : 256 * (s + 1)], a256[:], sc(1, j, s)
                    )
                b = jj * R
                # Final expansion: 4 x [128, 1024], split DVE/ACT.
                # (GpSimd measured 15 us per op here — keep it out.)
                nc.vector.tensor_scalar_mul(
                    ot[:, b : b + 1024], a1024[:], sc(0, j, 0)
                )
                nc.scalar.activation(
                    ot[:, b + 1024 : b + 2048],
                    a1024[:],
                    mybir.ActivationFunctionType.Copy,
                    scale=sc(0, j, 1),
                )
                nc.scalar.activation(
                    ot[:, b + 2048 : b + 3072],
                    a1024[:],
                    mybir.ActivationFunctionType.Copy,
                    scale=sc(0, j, 2),
                )
                nc.vector.tensor_scalar_mul(
                    ot[:, b + 3072 : b + 4096], a1024[:], sc(0, j, 3)
                )
            cols = t * JPAIR * R
            nc.sync.dma_start(out=out_v[:, cols : cols + JPAIR * R], in_=ot[:])

    nc.compile()
    return nc


def _get_nc():
    if "nc" not in _CACHE:
        _CACHE["nc"] = build_nc()
    return _CACHE["nc"]


def kernel(memberships):
    global LAST_RESULTS
    m = np.ascontiguousarray(np.asarray(memberships, dtype=np.float32))
    assert m.shape == (N_VARS, N_FULL, N_SETS), m.shape
    nc = _get_nc()
    shards = np.split(m, N_CORES, axis=1)
    in_maps = [{"memberships": np.ascontiguousarray(s)} for s in shards]
    res = run_bass_kernel_spmd(nc, in_maps, core_ids=list(range(N_CORES)))
    LAST_RESULTS = res
    return np.concatenate(
        [res.results[i]["out"] for i in range(N_CORES)], axis=0
    )
